# revision 41
# baseline (speedup 1.0000x reference)
"""Causal self-attention on Trainium2, tensor-parallel over heads across 8 NeuronCores.

Strategy (sharding_hint "tensor-parallel split the n_heads axis"):
  - Each core c owns heads {2c, 2c+1} == columns [128c, 128c+128) of Wq/Wk/Wv
    and rows [128c, 128c+128) of Wo.
  - Per core: QT/KT = (x @ W{q,k})^T in [feat, tok] layout, V in [tok, feat]
    layout with an appended ones column (denominator trick).
  - Scores are computed transposed ([k, q] layout) so exp(scoresT) feeds the
    PV matmul directly (lhsT = V_aug stationary, rhs = attnT streaming), which
    also yields the softmax denominators as row 64 of the PV output.
  - Normalization: reciprocal of the denominator row, broadcast across
    partitions with a K=1 matmul, multiply -> attnoutT [feat, tok].
  - Partial out-projection y_c = attnout_c @ Wo_c; host sums the 8 partials
    and adds bo.  (All-reduce done on host: gather/unshard step.)

Matmul inputs are bf16 (PSUM accumulation fp32): single-pass matmuls + FWL
weight loads, vs fp32's LOW_HIGH double pass.
"""

import sys

if "/opt/trn_rl_repo" not in sys.path:
    sys.path.insert(0, "/opt/trn_rl_repo")

from contextlib import ExitStack

import ml_dtypes
import numpy as np

import concourse.bass as bass
import concourse.mybir as mybir
import concourse.tile as tile

F32 = mybir.dt.float32
BF = mybir.dt.bfloat16
NPBF = ml_dtypes.bfloat16
EXP = mybir.ActivationFunctionType.Exp
LN = mybir.ActivationFunctionType.Ln
COPY = mybir.ActivationFunctionType.Copy

P = 128  # partition tile
HD = 64  # head dim
HC = 2  # heads per core (HC*HD == P)
WIN = 512  # token window (one PSUM bank of fp32)
MASK_VAL = -30000.0
N_WARM = 48  # PE warm-up matmuls (run under the x-load DMA shadow)


def _legalize_waits(nc):
    """This walrus build encodes at most ONE semaphore wait per instruction
    (setupSyncWait raises "Too many sync wait commands" otherwise).  Tile
    freely emits 2+ waits, so excess waits are moved onto injected same-engine
    NoOps (one wait each) directly before the instruction."""
    nop_id = 0
    for fn in nc.m.functions:
        for blk in fn.blocks:
            out = []
            for inst in blk.instructions:
                if type(inst).__name__ != "InstNoOp":
                    si = inst.sync_info
                    waits = list(si.on_wait or []) if si is not None else []
                    if len(waits) > 1:
                        for w in waits[1:]:
                            nop = mybir.InstNoOp(
                                name=f"nopw-{nop_id}",
                                engine=inst.engine,
                                ins=[],
                                outs=[],
                                sync_info=mybir.SyncInfo(on_wait=[w], on_update=[]),
                            )
                            nop_id += 1
                            out.append(nop)
                        si.on_wait = waits[:1]
                out.append(inst)
            blk.instructions[:] = out


def build_nc(B, T, D, n_cores, debug_dump=False, legalize=True):
    """Build the SPMD Bass program (same program all cores, per-core data)."""
    nj = D // P  # contraction tiles for projections
    n_win = T // WIN  # q windows per batch
    n_qt = T // P  # token tiles per batch
    M = B * T
    VW = 2 * P  # V_aug cols per token tile: per head [V(64) | one | zeros(63)]

    nc = bass.Bass("TRN2", target_bir_lowering=False, debug=False, num_devices=n_cores)
    if debug_dump:
        dq = nc.dram_tensor("dq", [P, M], F32, kind="ExternalOutput").ap()
        dk = nc.dram_tensor("dk", [P, M], F32, kind="ExternalOutput").ap()
        dv = nc.dram_tensor("dv", [P, B * (M // B // P) * VW], F32, kind="ExternalOutput").ap()
        da = nc.dram_tensor("da", [P, M], F32, kind="ExternalOutput").ap()

    xt = nc.dram_tensor("xt", [D, M], BF, kind="ExternalInput").ap()
    wq = nc.dram_tensor("wq", [P, D], BF, kind="ExternalInput").ap()
    wk = nc.dram_tensor("wk", [P, D], BF, kind="ExternalInput").ap()
    wv = nc.dram_tensor("wv", [P, D], BF, kind="ExternalInput").ap()
    wo = nc.dram_tensor("wo", [P, D], BF, kind="ExternalInput").ap()
    bq = nc.dram_tensor("bq", [1, P], BF, kind="ExternalInput").ap()
    bk = nc.dram_tensor("bk", [1, P], BF, kind="ExternalInput").ap()
    bv = nc.dram_tensor("bv", [1, P], BF, kind="ExternalInput").ap()
    msk = nc.dram_tensor("msk", [P, P], F32, kind="ExternalInput").ap()
    y = nc.dram_tensor("y", [M, D], BF, kind="ExternalOutput").ap()

    with tile.TileContext(nc) as tc, ExitStack() as ctx:
        const = ctx.enter_context(tc.tile_pool(name="const", bufs=1))
        xtp = ctx.enter_context(tc.tile_pool(name="xt", bufs=nj + 2))
        qkp = ctx.enter_context(tc.tile_pool(name="qk", bufs=2))
        vp = ctx.enter_context(tc.tile_pool(name="vaug", bufs=2))
        atp = ctx.enter_context(tc.tile_pool(name="attnT", bufs=6))
        aop = ctx.enter_context(tc.tile_pool(name="aoT", bufs=2))
        pvp = ctx.enter_context(tc.tile_pool(name="pvs", bufs=6))
        rcp = ctx.enter_context(tc.tile_pool(name="rc", bufs=6))
        yp = ctx.enter_context(tc.tile_pool(name="ysb", bufs=4))
        # PSUM budget (8 banks): sc 2x2-bank pairs + pv 2x1 + proj/y 2x1
        ps_sc = ctx.enter_context(tc.tile_pool(name="ps_sc", bufs=2, space="PSUM"))
        ps_pv = ctx.enter_context(tc.tile_pool(name="ps_pv", bufs=2, space="PSUM"))
        ps_proj = ctx.enter_context(tc.tile_pool(name="ps_proj", bufs=2, space="PSUM"))
        ps_proj = ps_sc
        ps_y = ps_sc

        # constants / weights
        wq_s = const.tile([P, D], BF, tag="wq")
        wk_s = const.tile([P, D], BF, tag="wk")
        wv_s = const.tile([P, D], BF, tag="wv")
        wo_s = const.tile([P, D], BF, tag="wo")
        bq_s = const.tile([1, P], BF, tag="bq")
        bk_s = const.tile([1, P], BF, tag="bk")
        bv_s = const.tile([1, P], BF, tag="bv")
        msk_s = const.tile([P, P], F32, tag="msk")
        ones_r = const.tile([1, WIN], BF, tag="ones")
        ones_f = const.tile([1, HD], F32, tag="onesf")
        warm_s = const.tile([P, WIN], BF, tag="warm")
        nc.vector.memset(ones_r[:, :], 1.0)
        nc.vector.memset(ones_f[:, :], 1.0)
        nc.vector.memset(warm_s[:, :], 1.0)
        nc.sync.dma_start(wq_s[:, :], wq[:, :])
        nc.sync.dma_start(wk_s[:, :], wk[:, :])
        nc.sync.dma_start(wv_s[:, :], wv[:, :])
        nc.sync.dma_start(wo_s[:, :], wo[:, :])
        nc.sync.dma_start(bq_s[:, :], bq[:, :])
        nc.sync.dma_start(bk_s[:, :], bk[:, :])
        nc.sync.dma_start(bv_s[:, :], bv[:, :])
        nc.sync.dma_start(msk_s[:, :], msk[:, :])

        # PE warm-up: dense dummy matmuls while the first x tiles stream in,
        # so the HAM clock gate reaches 8/8 before the real work starts.
        psw = ps_y.tile([P, WIN], F32, tag="sc")
        for i in range(N_WARM):
            nc.tensor.matmul(
                psw[:, :], warm_s[:, 0:P], warm_s[:, :], start=True, stop=True
            )

        for b in range(B):
            toff = b * T

            xts = []
            for j in range(nj):
                xt_t = xtp.tile([P, T], BF, tag="xt")
                nc.sync.dma_start(xt_t[:, :], xt[j * P : (j + 1) * P, toff : toff + T])
                xts.append(xt_t)

            # ---- QT / KT : [feat, tok], Q pre-scaled by 1/sqrt(HD) ----
            qt_s = qkp.tile([P, T], BF, tag="qt")
            kt_s = qkp.tile([P, T], BF, tag="kt")
            for w in range(n_win):
                ws = w * WIN
                psq = ps_proj.tile([P, WIN], F32, tag="sc")
                for j in range(nj):
                    nc.tensor.matmul(
                        psq[:, :],
                        wq_s[:, j * P : (j + 1) * P],
                        xts[j][:, ws : ws + WIN],
                        start=(j == 0),
                        stop=False,
                    )
                nc.tensor.matmul(
                    psq[:, :], bq_s[:, :], ones_r[:, :], start=False, stop=True
                )
                nc.vector.tensor_copy(qt_s[:, ws : ws + WIN], psq[:, :])
                psk = ps_proj.tile([P, WIN], F32, tag="sc")
                for j in range(nj):
                    nc.tensor.matmul(
                        psk[:, :],
                        wk_s[:, j * P : (j + 1) * P],
                        xts[j][:, ws : ws + WIN],
                        start=(j == 0),
                        stop=False,
                    )
                nc.tensor.matmul(
                    psk[:, :], bk_s[:, :], ones_r[:, :], start=False, stop=True
                )
                nc.vector.tensor_copy(kt_s[:, ws : ws + WIN], psk[:, :])

            # ---- V_aug : [tok, h0 V | 1 | h1 V | 1] per token tile ----
            vaug = vp.tile([P, n_qt * VW], BF, tag="vaug")
            nc.vector.memset(vaug[:, HD :: HD + 1], 1.0)  # ones cols at 64,129,194,...
            for t in range(n_qt):
                base = t * VW
                psv = ps_proj.tile([P, P], F32, tag="sc")
                for j in range(nj):
                    nc.tensor.matmul(
                        psv[:, :],
                        xts[j][:, t * P : (t + 1) * P],
                        wv_s[:, j * P : (j + 1) * P],
                        start=(j == 0),
                        stop=False,
                    )
                nc.tensor.matmul(
                    psv[:, :], ones_r[:, 0:P], bv_s[:, :], start=False, stop=True
                )
                nc.vector.tensor_copy(vaug[:, base : base + HD], psv[:, 0:HD])
                nc.vector.tensor_copy(
                    vaug[:, base + HD + 1 : base + 2 * HD + 1], psv[:, HD : 2 * HD]
                )

            # ---- attention: scoresT chunks [k-tile, q-window] -> exp -> PV ----
            aoT = aop.tile([P, T], BF, tag="aoT")

            def normalize(pvsb, rc, hp, ws):
                # aoT[h, w] = pv[0:HD] * recip(denom)-broadcast; traced one
                # window late so the PE never stalls on the DVE reciprocal.
                psb = ps_sc.tile([HD, WIN], F32, tag="sc")
                nc.tensor.matmul(
                    psb[:, :], ones_f[:, :], rc[:, :], start=True, stop=True
                )
                nc.vector.tensor_mul(
                    aoT[hp : hp + HD, ws : ws + WIN], pvsb[0:HD, :], psb[:, :]
                )

            # Heads are interleaved so their K=64 score matmuls sit adjacent in
            # the PE stream: disjoint row groups (rows 0-63 / 64-127) execute
            # concurrently in the array.  k tiles go two at a time: both score
            # chunks of a head land in one 2-bank PSUM tile, one exp per pair;
            # PV for pair p is traced after the scores of pair p+1 so the PE
            # never waits on the ACT exp.
            pend = []
            for w in range(n_win):
                ws = w * WIN
                njt = (ws + WIN) // P  # causal k tiles for this window
                pspv = [
                    ps_pv.tile([HD + 1, WIN], F32, tag="pv", name=f"pspv{_h}")
                    for _h in range(HC)
                ]

                def flush_pv(at, halves):
                    for h in range(HC):
                        for j, off, N, qstart in halves[h]:
                            vb = j * VW + h * (HD + 1)
                            nc.tensor.matmul(
                                pspv[h][:, qstart - ws : WIN],
                                vaug[:, vb : vb + HD + 1],
                                at[h][:, off : off + N],
                                start=(j == 0),
                                stop=(j == njt - 1),
                            )

                prev = None
                for j0 in range(0, njt, 2):
                    pss = [
                        ps_sc.tile([P, 2 * WIN], F32, tag="sc", name=f"pss{_h}")
                        for _h in range(HC)
                    ]
                    at = [
                        atp.tile([P, 2 * WIN], BF, tag="at", name=f"at{_h}")
                        for _h in range(HC)
                    ]
                    halves = [[] for _ in range(HC)]
                    off = [0] * HC
                    for j in (j0, j0 + 1):
                        if j >= njt:
                            continue
                        qstart = max(ws, j * P)
                        N = ws + WIN - qstart
                        for h in range(HC):
                            hp = h * HD
                            o = off[h]
                            if o and o + N > WIN:
                                o = WIN  # don't straddle a PSUM bank
                            nc.tensor.matmul(
                                pss[h][:, o : o + N],
                                kt_s[hp : hp + HD, j * P : (j + 1) * P],
                                qt_s[hp : hp + HD, qstart : qstart + N],
                                start=True,
                                stop=True,
                                tile_position=(hp, 0),
                            )
                            halves[h].append((j, o, N, qstart))
                            off[h] = o + N
                        if j * P >= ws:  # chunk starts on the diagonal
                            for h in range(HC):
                                o = halves[h][-1][1]
                                nc.vector.tensor_add(
                                    pss[h][:, o : o + P], pss[h][:, o : o + P],
                                    msk_s[:, :],
                                )
                    for h in range(HC):
                        width = halves[h][-1][1] + halves[h][-1][2]
                        nc.scalar.activation(
                            at[h][:, 0:width], pss[h][:, 0:width], EXP
                        )
                    if prev is not None:
                        flush_pv(*prev)
                    prev = (at, halves)
                flush_pv(*prev)
                for h in range(HC):
                    pvsb = pvp.tile([HD + 1, WIN], F32, tag="pvs")
                    nc.vector.tensor_copy(pvsb[:, :], pspv[h][0 : HD + 1, :])
                    rc = rcp.tile([1, WIN], F32, tag="rc")
                    nc.vector.reciprocal(rc[:, :], pspv[h][HD : HD + 1, :])
                    pend.append((pvsb, rc, h * HD, ws))
                    while len(pend) > 2:
                        normalize(*pend.pop(0))
            for args in pend:
                normalize(*args)
            pend = []

            if debug_dump:
                nc.sync.dma_start(dq[:, toff : toff + T], qt_s[:, :])
                nc.sync.dma_start(dk[:, toff : toff + T], kt_s[:, :])
                nc.sync.dma_start(
                    dv[:, b * n_qt * VW : (b + 1) * n_qt * VW], vaug[:, :]
                )
                nc.sync.dma_start(da[:, toff : toff + T], aoT[:, :])

            # ---- partial out-projection ----
            for t in range(n_qt):
                for ui, u0 in enumerate(range(0, D, WIN)):
                    N = min(WIN, D - u0)
                    psy = ps_y.tile([P, WIN], F32, tag="sc")
                    nc.tensor.matmul(
                        psy[:, 0:N],
                        aoT[:, t * P : (t + 1) * P],
                        wo_s[:, u0 : u0 + N],
                        start=True,
                        stop=True,
                    )
                    ysb = yp.tile([P, WIN], F32, tag="ysb")
                    nc.vector.tensor_copy(ysb[:, 0:N], psy[:, 0:N])
                    nc.sync.dma_start(
                        y[toff + t * P : toff + (t + 1) * P, u0 : u0 + N], ysb[:, 0:N]
                    )
    if legalize:
        _legalize_waits(nc)
    return nc


def make_in_maps(x, Wq, bq, Wk, bk, Wv, bv, Wo, n_cores):
    x = np.asarray(x, dtype=np.float32)
    Bb, Tt, Dd = x.shape
    M = Bb * Tt
    xt = np.ascontiguousarray(x.reshape(M, Dd).T.astype(NPBF))
    mask = np.where(
        np.arange(P)[:, None] > np.arange(P)[None, :], MASK_VAL, 0.0
    ).astype(np.float32)

    def wslice(W, c, scale=1.0):
        Wc = np.asarray(W, np.float32)[:, c * P : (c + 1) * P] * np.float32(scale)
        return np.ascontiguousarray(
            Wc.reshape(Dd // P, P, P).transpose(1, 0, 2).reshape(P, Dd).astype(NPBF)
        )

    qscale = 1.0 / np.sqrt(HD)
    in_maps = []
    for c in range(n_cores):
        cs = slice(c * P, (c + 1) * P)
        in_maps.append(
            {
                "xt": xt,
                "wq": wslice(Wq, c, qscale),
                "wk": wslice(Wk, c),
                "wv": wslice(Wv, c),
                "wo": np.ascontiguousarray(
                    np.asarray(Wo, np.float32)[cs, :].astype(NPBF)
                ),
                "bq": np.ascontiguousarray(
                    (np.asarray(bq, np.float32)[cs] * np.float32(qscale))
                    .reshape(1, P)
                    .astype(NPBF)
                ),
                "bk": np.ascontiguousarray(
                    np.asarray(bk, np.float32)[cs].reshape(1, P).astype(NPBF)
                ),
                "bv": np.ascontiguousarray(
                    np.asarray(bv, np.float32)[cs].reshape(1, P).astype(NPBF)
                ),
                "msk": mask,
            }
        )
    return in_maps


_NC_CACHE = {}


def get_nc(B, T, D, n_cores):
    key = (B, T, D, n_cores)
    if key not in _NC_CACHE:
        _NC_CACHE[key] = build_nc(B, T, D, n_cores)
    return _NC_CACHE[key]


def kernel(**inputs):
    from concourse.bass_utils import run_bass_kernel_spmd

    x = np.asarray(inputs["x"], np.float32)
    Bb, Tt, Dd = x.shape
    n_cores = 8
    nc = get_nc(Bb, Tt, Dd, n_cores)
    in_maps = make_in_maps(
        x,
        inputs["Wq"],
        inputs["bq"],
        inputs["Wk"],
        inputs["bk"],
        inputs["Wv"],
        inputs["bv"],
        inputs["Wo"],
        n_cores,
    )
    res = run_bass_kernel_spmd(nc, in_maps, core_ids=list(range(n_cores)))
    y = np.zeros((Bb * Tt, Dd), dtype=np.float64)
    for r in res.results:
        y += r["y"].astype(np.float64)
    y += np.asarray(inputs["bo"], np.float64)[None, :]
    return y.reshape(Bb, Tt, Dd).astype(np.float32)
